# revision 35
# baseline (speedup 1.0000x reference)
"""HardAttentionMemoryAE Trainium2 kernel (v2: host-transposed bf16 I/O).

Data-parallel over 8 NeuronCores: x sharded along batch, weights + 50x128
memory bank replicated.

Key design vs the v1 (PE-transpose) kernel:
- x is transposed + bf16-cast on the HOST into the exact per-slab SBUF
  layout ([128, 6*512] chunk-major + [16, 512] tail), so the 28 PE
  transposes + 7 PSUM->SBUF copies per slab disappear entirely.
- All weights pre-packed/cast to bf16 on the host (W1/W2/W4 chunk layouts,
  mem_norm^T, W3m = memory @ W3). No device-side setup matmuls.
- bf16 output (y dram is bf16, host upcasts): halves output HBM traffic.
- Every matmul is bf16 both sides -> FWL fast weight loads; the final
  matmul runs k-outer so each dT stationary chunk is loaded once (8
  LDWEIGHTS/slab instead of 16).
- Scalar engine keeps only Exp (softmax, accum denominator) and Tanh
  (sigmoid(v) = 0.5*tanh(0.5 v)+0.5) -> one table set, no reloads.
  ReLU+bias runs on DVE (tensor_scalar add+max vs PSUM), z bias on
  GpSimd, row-norm square+reduce fused into DVE tensor_tensor_reduce.
- 1/sqrt(||z||^2) via int-bit-trick seed + Newton on GpSimd; 1/||z|| is
  folded into the Exp activation's per-partition scale operand; top-k
  thresholding runs on raw sims (scale-invariant).
- PSUM: pbig 2x1 banks (ph/pd), pxp 2x2 banks (final), pmisc 2x1 banks
  (pz/zrm/psim/pat in dataflow order).
"""
import numpy as np
import ml_dtypes
import concourse.bass as bass
import concourse.mybir as mybir
from concourse import bacc
from concourse.tile import TileContext
from concourse.masks import make_identity
from concourse.bass_utils import run_bass_kernel_spmd

F32 = mybir.dt.float32
F32R = mybir.dt.float32r
BF16 = mybir.dt.bfloat16
I32 = mybir.dt.int32
AF = mybir.ActivationFunctionType
ALU = mybir.AluOpType

B_FULL = 65536
D = 784          # input dim
E = 128          # embed dim
M = 50           # memory slots
H = 256          # hidden
N_CORES = 8
SLAB = 512       # rows per slab (4 row-tiles of 128)
NHALF = 392      # final matmul N split (per PSUM bank half)
PCH = 112        # input-dim chunk: 784 = 7*112 exactly (7 x 112-partition)

TRACE = False    # set by test harness for profiling runs
NEWTON_STEPS = 1
NPBF = np.dtype(ml_dtypes.bfloat16)


def _build(rows: int, n_cores: int, bias_mm: bool):
    nc = bacc.Bacc(
        "TRN2", target_bir_lowering=False, debug=False,
        enable_asserts=True, num_devices=n_cores
    )
    n_slabs = rows // SLAB
    xm = nc.dram_tensor("xmain", [n_slabs, PCH, 7 * SLAB], BF16,
                        kind="ExternalInput")
    W1m = nc.dram_tensor("W1sb", [PCH, 7 * H], BF16, kind="ExternalInput")
    W2d = nc.dram_tensor("W2sb", [128, H], BF16, kind="ExternalInput")
    mnTd = nc.dram_tensor("mnT", [E, M], BF16, kind="ExternalInput")
    W3md = nc.dram_tensor("W3msb", [M, H], BF16, kind="ExternalInput")
    W4d = nc.dram_tensor("W4sb", [128, 2 * D], BF16, kind="ExternalInput")
    b1d = nc.dram_tensor("b1c", [128, 2], F32, kind="ExternalInput")
    b2d = nc.dram_tensor("b2c", [128, 1], F32, kind="ExternalInput")
    b3d = nc.dram_tensor("b3c", [128, 2], F32, kind="ExternalInput")
    if bias_mm:
        b4d = nc.dram_tensor("b4row", [1, D], F32, kind="ExternalInput")
    y = nc.dram_tensor("y", [rows, D], BF16, kind="ExternalOutput")

    y_r = y[:].rearrange("(s t p) c -> s p t c", p=128, t=4)

    with TileContext(nc) as tc:
        with (
            tc.tile_pool(name="const", bufs=1) as cpool,
            tc.tile_pool(name="xa", bufs=3) as xa_pool,
            tc.tile_pool(name="hT", bufs=2) as hT_pool,
            tc.tile_pool(name="zT", bufs=2) as zT_pool,
            tc.tile_pool(name="small", bufs=2) as sm_pool,
            tc.tile_pool(name="xout", bufs=2) as xo_pool,
            tc.tile_pool(name="pbig", bufs=2, space="PSUM") as pbig,
            tc.tile_pool(name="pxp", bufs=4, space="PSUM") as pxp,
            tc.tile_pool(name="pmisc", bufs=2, space="PSUM") as pmisc,
        ):
            st = {}   # slab index -> dict of live tiles

            def emit_dma_in(s):
                d = st.setdefault(s, {})
                d["xa"] = xa_pool.tile([PCH, 7 * SLAB], BF16, tag="xa",
                                       name=f"xa_{s}")
                nc.sync.dma_start(d["xa"][:], xm[s])

            # prefetch the first two x slabs before the weight DMAs so the
            # first p1 matmuls start as early as possible
            emit_dma_in(0)
            emit_dma_in(1)

            # ---------------- one-time setup (DMAs only) ----------------
            ident_f = cpool.tile([128, 128], F32)
            make_identity(nc, ident_f[:])
            ident_b = cpool.tile([128, 128], BF16)
            nc.vector.tensor_copy(ident_b[:], ident_f[:])

            ones_col = cpool.tile([128, 1], BF16)
            nc.vector.memset(ones_col[:], 1.0)

            W1sb = cpool.tile([PCH, 7 * H], BF16)
            nc.gpsimd.dma_start(W1sb[:], W1m[:])
            W2sb = cpool.tile([128, H], BF16)
            nc.gpsimd.dma_start(W2sb[:], W2d[:])
            mnT = cpool.tile([E, M], BF16)
            nc.gpsimd.dma_start(mnT[:], mnTd[:])
            W3msb = cpool.tile([M, H], BF16)
            nc.gpsimd.dma_start(W3msb[:], W3md[:])
            W4sb = cpool.tile([128, 2 * D], BF16)
            nc.gpsimd.dma_start(W4sb[:], W4d[:])
            b1sb = cpool.tile([128, 2], F32)
            nc.gpsimd.dma_start(b1sb[:], b1d[:])
            b2sb = cpool.tile([128, 1], F32)
            nc.gpsimd.dma_start(b2sb[:], b2d[:])
            b3sb = cpool.tile([128, 2], F32)
            nc.gpsimd.dma_start(b3sb[:], b3d[:])
            if bias_mm:
                b4row = cpool.tile([1, D], F32R)
                nc.gpsimd.dma_start(b4row[:], b4d[:])
                ones_row = cpool.tile([1, 128], F32R)
                onesr_f = cpool.tile([1, 128], F32)
                nc.vector.memset(onesr_f[:], 1.0)
                nc.scalar.copy(ones_row[:], onesr_f[:])

            # ---------------- per-slab stage emitters ----------------
            def emit_p1(s, m):
                d = st[s]
                if "hT" not in d:
                    d["hT"] = hT_pool.tile([128, 1024], BF16, tag="hT",
                                           name=f"hT_{s}")
                ph = pbig.tile([128, 512], F32, tag="big", name=f"ph{m}_{s}")
                for c in range(7):
                    nc.tensor.matmul(
                        ph[:],
                        W1sb[:, c * H + m * 128: c * H + m * 128 + 128],
                        d["xa"][:, c * SLAB:(c + 1) * SLAB],
                        start=(c == 0), stop=(c == 6),
                    )
                # relu(ph + b1) on DVE (ACT is kept clear for the topk exps
                # so the attention chain isn't queue-blocked)
                nc.vector.tensor_scalar(
                    out=d["hT"][:, m * 512:(m + 1) * 512], in0=ph[:],
                    scalar1=b1sb[:, m:m + 1], scalar2=0.0,
                    op0=ALU.add, op1=ALU.max,
                )

            def emit_p2(s):
                d = st[s]
                pz = pmisc.tile([128, 512], F32, tag="misc", name=f"pz_{s}")
                for m in range(2):
                    nc.tensor.matmul(
                        pz[:], W2sb[:, m * 128:(m + 1) * 128],
                        d["hT"][:, m * 512:(m + 1) * 512],
                        start=(m == 0), stop=(m == 1),
                    )
                d["zT"] = zT_pool.tile([128, SLAB], BF16, tag="zT",
                                       name=f"zT_{s}")
                # z = pz + b2 on DVE (ACT keeps only tanh/exp/copy)
                nc.vector.tensor_scalar(
                    out=d["zT"][:], in0=pz[:], scalar1=b2sb[:, 0:1],
                    scalar2=None, op0=ALU.add,
                )

            def emit_norm(s):
                # squared z on GpSimd (SBUF->SBUF); row sums come from PE
                # matmuls against a ones column (emit_sims)
                d = st[s]
                d["zsqT"] = sm_pool.tile([128, 512], BF16, tag="zsqT",
                                         name=f"zsqT_{s}")
                nc.gpsimd.tensor_tensor(d["zsqT"][:], d["zT"][:], d["zT"][:],
                                        ALU.mult)

            def emit_sims(s):
                d = st[s]
                psim = pmisc.tile([128, 512], F32, tag="misc",
                                  name=f"psim_{s}")
                for t in range(4):
                    nc.tensor.matmul(
                        psim[:, t * M:(t + 1) * M],
                        d["zT"][:, t * 128:(t + 1) * 128], mnT[:],
                        start=True, stop=True,
                    )
                for t in range(4):
                    # ||z||^2 for row tile t -> psim col 200+t (needs the
                    # GpSimd square; emitted after the sims MMs so the PE
                    # isn't blocked on it)
                    nc.tensor.matmul(
                        psim[:, 4 * M + t:4 * M + t + 1],
                        d["zsqT"][:, t * 128:(t + 1) * 128], ones_col[:],
                        start=True, stop=True,
                    )
                d["psim"] = psim
                # inv = 1/sqrt(nsq): magic-constant seed + Newton steps.
                # seed/h read PSUM on DVE; Newton runs on GpSimd from SBUF.
                nsq = psim[:, 4 * M:4 * M + 4]
                seed_i = sm_pool.tile([128, 4], I32, tag="seed_i",
                                      name=f"seed_i_{s}")
                nc.vector.tensor_scalar(
                    out=seed_i[:], in0=nsq.bitcast(I32),
                    scalar1=1, scalar2=None, op0=ALU.logical_shift_right,
                )
                y0_i = sm_pool.tile([128, 4], I32, tag="y0_i",
                                    name=f"y0_i_{s}")
                nc.vector.tensor_scalar(
                    out=y0_i[:], in0=seed_i[:],
                    scalar1=-1, scalar2=0x5F3759DF, op0=ALU.mult,
                    op1=ALU.add,
                )
                h = sm_pool.tile([128, 4], F32, tag="h", name=f"h_{s}")
                nc.vector.tensor_scalar(
                    out=h[:], in0=nsq, scalar1=0.5, scalar2=1e-30,
                    op0=ALU.mult, op1=ALU.max,
                )
                ycur = y0_i[:].bitcast(F32)
                for it in range(NEWTON_STEPS):
                    a = sm_pool.tile([128, 4], F32, tag=f"nta{it}",
                                     name=f"nta{it}_{s}")
                    nc.gpsimd.tensor_tensor(a[:], ycur, ycur, ALU.mult)
                    b_ = sm_pool.tile([128, 4], F32, tag=f"ntb{it}",
                                      name=f"ntb{it}_{s}")
                    nc.gpsimd.tensor_tensor(b_[:], a[:], h[:], ALU.mult)
                    c_ = sm_pool.tile([128, 4], F32, tag=f"ntc{it}",
                                      name=f"ntc{it}_{s}")
                    nc.gpsimd.tensor_scalar(
                        out=c_[:], in0=b_[:], scalar1=-1.0, scalar2=1.5,
                        op0=ALU.mult, op1=ALU.add,
                    )
                    ynext = sm_pool.tile([128, 4], F32, tag=f"nty{it}",
                                         name=f"nty{it}_{s}")
                    nc.gpsimd.tensor_tensor(ynext[:], ycur, c_[:], ALU.mult)
                    ycur = ynext[:]
                d["invcol"] = ycur

            def emit_topk(s):
                d = st[s]
                # cosine sims = raw sims * 1/||z|| (DVE, from PSUM); top-k
                # selection is scale-invariant so thresholding the scaled
                # sims matches the reference
                simsb = sm_pool.tile([128, 4 * M], F32, tag="simsb",
                                     name=f"simsb_{s}")
                for t in range(4):
                    nc.vector.tensor_scalar_mul(
                        simsb[:, t * M:(t + 1) * M],
                        d["psim"][:, t * M:(t + 1) * M],
                        d["invcol"][:, t:t + 1],
                    )
                m8 = sm_pool.tile([128, 32], F32, tag="m8", name=f"m8_{s}")
                msk = sm_pool.tile([128, 4 * M], F32, tag="msk",
                                   name=f"msk_{s}")
                for t in range(4):
                    nc.vector.max(m8[:, t * 8:(t + 1) * 8],
                                  simsb[:, t * M:(t + 1) * M])
                    nc.vector.scalar_tensor_tensor(
                        out=msk[:, t * M:(t + 1) * M],
                        in0=simsb[:, t * M:(t + 1) * M],
                        scalar=m8[:, t * 8 + 4:t * 8 + 5],
                        in1=simsb[:, t * M:(t + 1) * M],
                        op0=ALU.is_ge, op1=ALU.mult,
                    )
                # per-row-tile exp -> denominator accum -> recip -> scale so
                # tile t's tail overlaps tile t+1's exp (exp(0)=1 for masked
                # entries, as in reference); final scale on GpSimd
                pexp = sm_pool.tile([128, 4 * M], F32, tag="pexp",
                                    name=f"pexp_{s}")
                den = sm_pool.tile([128, 4], F32, tag="den", name=f"den_{s}")
                rden = sm_pool.tile([128, 4], F32, tag="rden",
                                    name=f"rden_{s}")
                attn = sm_pool.tile([128, 4 * M], BF16, tag="attn",
                                    name=f"attn_{s}")
                for t in range(4):
                    nc.scalar.activation(
                        pexp[:, t * M:(t + 1) * M], msk[:, t * M:(t + 1) * M],
                        AF.Exp, accum_out=den[:, t:t + 1],
                    )
                    nc.vector.reciprocal(rden[:, t:t + 1], den[:, t:t + 1])
                    # normalization as an ACT Copy with per-partition scale
                    # (AP scalars aren't legal on GpSimd)
                    nc.scalar.activation(
                        attn[:, t * M:(t + 1) * M],
                        pexp[:, t * M:(t + 1) * M],
                        AF.Copy, scale=rden[:, t:t + 1],
                    )
                d["attn"] = attn

            def emit_pat(s):
                d = st[s]
                pat = pmisc.tile([128, 512], BF16, tag="misc",
                                 name=f"pat_{s}")
                for t in range(4):
                    nc.tensor.transpose(
                        pat[:M, t * 128:(t + 1) * 128],
                        d["attn"][:, t * M:(t + 1) * M], ident_b[:],
                    )
                attnT = sm_pool.tile([M, SLAB], BF16, tag="attnT",
                                     name=f"attnT_{s}")
                nc.vector.tensor_copy(attnT[:], pat[:M, :])
                d["attnT"] = attnT

            def emit_dec(s):
                d = st[s]
                d["dT"] = hT_pool.tile([128, 1024], BF16, tag="dT",
                                       name=f"dT_{s}")
                for m in range(2):
                    pd = pbig.tile([128, 512], F32, tag="big",
                                   name=f"pd{m}_{s}")
                    nc.tensor.matmul(
                        pd[:], W3msb[:, m * 128:(m + 1) * 128], d["attnT"][:],
                        start=True, stop=True,
                    )
                    # relu(pd + b3) on DVE -> bf16 dT
                    nc.vector.tensor_scalar(
                        out=d["dT"][:, m * 512:(m + 1) * 512], in0=pd[:],
                        scalar1=b3sb[:, m:m + 1], scalar2=0.0,
                        op0=ALU.add, op1=ALU.max,
                    )

            def emit_final(s, tiles):
                d = st[s]
                if "xo" not in d:
                    d["xo"] = xo_pool.tile([128, 4, D], BF16, tag="xo",
                                           name=f"xo_{s}")
                for t in tiles:
                    # two 1-bank PSUM tiles per row tile (nh halves); each
                    # dT stationary chunk loaded once for both halves
                    pxs = [pxp.tile([128, 512], F32, tag="x",
                                    name=f"px{t}{nh}_{s}") for nh in range(2)]
                    if bias_mm:
                        for nh in range(2):
                            nc.tensor.matmul(
                                pxs[nh][:, :NHALF], ones_row[:],
                                b4row[:, nh * NHALF:(nh + 1) * NHALF],
                                start=True, stop=False,
                            )
                    for k in range(2):
                        for nh in range(2):
                            nc.tensor.matmul(
                                pxs[nh][:, :NHALF],
                                d["dT"][:, k * 512 + t * 128:
                                        k * 512 + t * 128 + 128],
                                W4sb[:, k * D + nh * NHALF:
                                     k * D + (nh + 1) * NHALF],
                                start=(k == 0 and not bias_mm), stop=(k == 1),
                            )
                    # sigmoid(v) = 0.5*tanh(0.5 v)+0.5; tanh on ACT, the
                    # affine fixup runs later (emit_affine) on GpSimd
                    for nh in range(2):
                        nc.scalar.activation(
                            d["xo"][:, t, nh * NHALF:(nh + 1) * NHALF],
                            pxs[nh][:, :NHALF], AF.Tanh, scale=0.5,
                        )

            def emit_affine(s, tiles):
                d = st[s]
                for t in tiles:
                    nc.gpsimd.tensor_scalar(
                        out=d["xo"][:, t, :], in0=d["xo"][:, t, :],
                        scalar1=0.5, scalar2=0.5,
                        op0=ALU.mult, op1=ALU.add,
                    )

            def emit_out(s):
                nc.sync.dma_start(y_r[s], st[s]["xo"][:])
                del st[s]

            def emit_keepalive(s):
                # dep-free 1-column matmul into an unused psim column: keeps
                # the PE HAM activity window from going idle at the block
                # boundary (re-throttle to 1.2GHz costs ~2.5us/slab)
                if s in st and "psim" in st[s]:
                    nc.tensor.matmul(
                        st[s]["psim"][0:1, 205:206], ones_col[0:1, :],
                        ones_col[0:1, :], start=True, stop=True,
                    )

            # ------------- depth-3 software-pipelined slab loop -------------
            # Block s runs: encoder of slab s (PE), topk chain of s-1
            # (DVE/ACT), attention tail + decoder of s-2 (PE). The topk
            # chain thus has a full block of PE work to hide behind.
            for s in range(n_slabs):
                if s >= 2:
                    emit_pat(s - 2)
                emit_keepalive(s - 1)
                emit_p1(s, 0)
                emit_p1(s, 1)
                if s >= 2:
                    emit_dec(s - 2)
                    emit_final(s - 2, [0, 1])
                emit_p2(s)
                emit_norm(s)
                if s >= 1:
                    emit_topk(s - 1)
                emit_sims(s)
                if s + 2 < n_slabs:
                    emit_dma_in(s + 2)
                if s >= 2:
                    emit_final(s - 2, [2, 3])
                    emit_affine(s - 2, [0, 1, 2, 3])
                emit_keepalive(s)
                if s >= 2:
                    emit_out(s - 2)
            n = n_slabs
            emit_topk(n - 1)
            emit_pat(n - 2)
            emit_dec(n - 2)
            emit_final(n - 2, [0, 1])
            emit_pat(n - 1)
            emit_dec(n - 1)
            emit_final(n - 2, [2, 3])
            emit_affine(n - 2, [0, 1, 2, 3])
            emit_out(n - 2)
            emit_final(n - 1, [0, 1, 2, 3])
            emit_affine(n - 1, [0, 1, 2, 3])
            emit_out(n - 1)

    nc.finalize()
    return nc


_cache: dict = {}


def _get_nc(rows: int, n_cores: int, bias_mm: bool):
    key = (rows, n_cores, bias_mm)
    if key not in _cache:
        _cache[key] = _build(rows, n_cores, bias_mm)
    return _cache[key]


def kernel(**inputs):
    x = np.asarray(inputs["x"], dtype=np.float32)
    rows = x.shape[0]
    n_cores = N_CORES
    rows_pc = rows // n_cores
    n_slabs = rows_pc // SLAB
    b4 = np.asarray(inputs["b4"], np.float32)
    bias_mm = not np.allclose(b4, 0.0)
    nc = _get_nc(rows_pc, n_cores, bias_mm)

    W1 = np.asarray(inputs["W1"], np.float32)
    W2 = np.asarray(inputs["W2"], np.float32)
    W3 = np.asarray(inputs["W3"], np.float32)
    W4 = np.asarray(inputs["W4"], np.float32)
    mem = np.asarray(inputs["memory"], np.float32)
    b1 = np.asarray(inputs["b1"], np.float32)
    b2 = np.asarray(inputs["b2"], np.float32)
    b3 = np.asarray(inputs["b3"], np.float32)

    # pre-packed weight layouts (see _build)
    W1sb = np.ascontiguousarray(
        W1.reshape(7, PCH, H).transpose(1, 0, 2).reshape(PCH, 7 * H)
    ).astype(NPBF)
    W2sb = np.ascontiguousarray(
        W2.reshape(2, 128, E).transpose(1, 0, 2).reshape(128, H)
    ).astype(NPBF)
    nrm = np.sqrt((mem * mem).sum(axis=1, keepdims=True))
    mn = mem / np.maximum(nrm, 1e-12)
    mnT = np.ascontiguousarray(mn.T).astype(NPBF)               # [128, 50]
    W3m = (mem.astype(np.float64) @ W3.astype(np.float64)).astype(NPBF)
    W4sb = np.ascontiguousarray(
        W4.reshape(2, 128, D).transpose(1, 0, 2).reshape(128, 2 * D)
    ).astype(NPBF)
    b1c = np.ascontiguousarray(b1.reshape(2, 128).T)            # [128, 2]
    b2c = np.ascontiguousarray(b2.reshape(128, 1))
    b3c = np.ascontiguousarray(b3.reshape(2, 128).T)

    weights = {
        "W1sb": W1sb, "W2sb": W2sb, "mnT": mnT,
        "W3msb": W3m, "W4sb": W4sb, "b1c": b1c, "b2c": b2c, "b3c": b3c,
    }
    if bias_mm:
        weights["b4row"] = np.ascontiguousarray(b4.reshape(1, D))

    # per-core x, host-transposed into the slab layout:
    # xmain[s, p, c*512 + r] = x[s*512 + r, c*112 + p],  c in [0, 7)
    xb16 = x.astype(NPBF)
    in_maps = []
    for c in range(n_cores):
        xc = xb16[c * rows_pc:(c + 1) * rows_pc].reshape(n_slabs, SLAB, D)
        xmain = np.ascontiguousarray(
            xc.reshape(n_slabs, SLAB, 7, PCH).transpose(0, 3, 2, 1)
        ).reshape(n_slabs, PCH, 7 * SLAB)
        in_maps.append({"xmain": xmain, **weights})

    res = run_bass_kernel_spmd(
        nc, in_maps, list(range(n_cores)), trace=TRACE
    )
    kernel.last_result = res
    y = np.concatenate([np.asarray(res.results[c]["y"])
                        for c in range(n_cores)], axis=0)
    return y.astype(np.float32)


# revision 41
# speedup vs baseline: 1.2222x; 1.2222x over previous
"""HardAttentionMemoryAE Trainium2 kernel (v2: host-transposed bf16 I/O).

Data-parallel over 8 NeuronCores: x sharded along batch, weights + 50x128
memory bank replicated.

Key design vs the v1 (PE-transpose) kernel:
- x is transposed + bf16-cast on the HOST into the exact per-slab SBUF
  layout ([128, 6*512] chunk-major + [16, 512] tail), so the 28 PE
  transposes + 7 PSUM->SBUF copies per slab disappear entirely.
- All weights pre-packed/cast to bf16 on the host (W1/W2/W4 chunk layouts,
  mem_norm^T, W3m = memory @ W3). No device-side setup matmuls.
- bf16 output (y dram is bf16, host upcasts): halves output HBM traffic.
- Every matmul is bf16 both sides -> FWL fast weight loads; the final
  matmul runs k-outer so each dT stationary chunk is loaded once (8
  LDWEIGHTS/slab instead of 16).
- Scalar engine keeps only Exp (softmax, accum denominator) and Tanh
  (sigmoid(v) = 0.5*tanh(0.5 v)+0.5) -> one table set, no reloads.
  ReLU+bias runs on DVE (tensor_scalar add+max vs PSUM), z bias on
  GpSimd, row-norm square+reduce fused into DVE tensor_tensor_reduce.
- 1/sqrt(||z||^2) via int-bit-trick seed + Newton on GpSimd; 1/||z|| is
  folded into the Exp activation's per-partition scale operand; top-k
  thresholding runs on raw sims (scale-invariant).
- PSUM: pbig 2x1 banks (ph/pd), pxp 2x2 banks (final), pmisc 2x1 banks
  (pz/zrm/psim/pat in dataflow order).
"""
import numpy as np
import ml_dtypes
import concourse.bass as bass
import concourse.mybir as mybir
from concourse import bacc
from concourse.tile import TileContext
from concourse.masks import make_identity
from concourse.bass_utils import run_bass_kernel_spmd

F32 = mybir.dt.float32
F32R = mybir.dt.float32r
BF16 = mybir.dt.bfloat16
I32 = mybir.dt.int32
AF = mybir.ActivationFunctionType
ALU = mybir.AluOpType

B_FULL = 65536
D = 784          # input dim
E = 128          # embed dim
M = 50           # memory slots
H = 256          # hidden
N_CORES = 8
SLAB = 512       # rows per slab (4 row-tiles of 128)
NHALF = 392      # final matmul N split (per PSUM bank half)
PCH = 112        # input-dim chunk: 784 = 7*112 exactly (7 x 112-partition)

TRACE = False    # set by test harness for profiling runs
NEWTON_STEPS = 1
NPBF = np.dtype(ml_dtypes.bfloat16)


def _build(rows: int, n_cores: int, bias_mm: bool):
    nc = bacc.Bacc(
        "TRN2", target_bir_lowering=False, debug=False,
        enable_asserts=True, num_devices=n_cores
    )
    n_slabs = rows // SLAB
    xm = nc.dram_tensor("xmain", [n_slabs, PCH, 7 * SLAB], BF16,
                        kind="ExternalInput")
    W1m = nc.dram_tensor("W1sb", [PCH, 7 * H], BF16, kind="ExternalInput")
    W2d = nc.dram_tensor("W2sb", [128, H], BF16, kind="ExternalInput")
    mnTd = nc.dram_tensor("mnT", [E, M], BF16, kind="ExternalInput")
    W3md = nc.dram_tensor("W3msb", [M, H], BF16, kind="ExternalInput")
    W4d = nc.dram_tensor("W4sb", [128, 2 * D], BF16, kind="ExternalInput")
    b1d = nc.dram_tensor("b1c", [128, 2], F32, kind="ExternalInput")
    b2d = nc.dram_tensor("b2c", [128, 1], F32, kind="ExternalInput")
    b3d = nc.dram_tensor("b3c", [128, 2], F32, kind="ExternalInput")
    if bias_mm:
        b4d = nc.dram_tensor("b4row", [1, D], F32, kind="ExternalInput")
    y = nc.dram_tensor("y", [rows, D], BF16, kind="ExternalOutput")

    y_r = y[:].rearrange("(s t p) c -> s p t c", p=128, t=4)

    with TileContext(nc) as tc:
        with (
            tc.tile_pool(name="const", bufs=1) as cpool,
            tc.tile_pool(name="xa", bufs=3) as xa_pool,
            tc.tile_pool(name="hT", bufs=2) as hT_pool,
            tc.tile_pool(name="zT", bufs=2) as zT_pool,
            tc.tile_pool(name="small", bufs=2) as sm_pool,
            tc.tile_pool(name="xout", bufs=2) as xo_pool,
            tc.tile_pool(name="pbig", bufs=2, space="PSUM") as pbig,
            tc.tile_pool(name="pxp", bufs=4, space="PSUM") as pxp,
            tc.tile_pool(name="pmisc", bufs=2, space="PSUM") as pmisc,
        ):
            st = {}   # slab index -> dict of live tiles

            def emit_dma_in(s):
                d = st.setdefault(s, {})
                d["xa"] = xa_pool.tile([PCH, 7 * SLAB], BF16, tag="xa",
                                       name=f"xa_{s}")
                nc.sync.dma_start(d["xa"][:], xm[s])

            # prefetch the first two x slabs before the weight DMAs so the
            # first p1 matmuls start as early as possible
            emit_dma_in(0)
            emit_dma_in(1)

            # ---------------- one-time setup (DMAs only) ----------------
            ident_f = cpool.tile([128, 128], F32)
            make_identity(nc, ident_f[:])
            ident_b = cpool.tile([128, 128], BF16)
            nc.vector.tensor_copy(ident_b[:], ident_f[:])

            ones_col = cpool.tile([128, 1], BF16)
            nc.vector.memset(ones_col[:], 1.0)

            W1sb = cpool.tile([PCH, 7 * H], BF16)
            nc.gpsimd.dma_start(W1sb[:], W1m[:])
            W2sb = cpool.tile([128, H], BF16)
            nc.gpsimd.dma_start(W2sb[:], W2d[:])
            mnT = cpool.tile([E, M], BF16)
            nc.gpsimd.dma_start(mnT[:], mnTd[:])
            W3msb = cpool.tile([M, H], BF16)
            nc.gpsimd.dma_start(W3msb[:], W3md[:])
            W4sb = cpool.tile([128, 2 * D], BF16)
            nc.gpsimd.dma_start(W4sb[:], W4d[:])
            b1sb = cpool.tile([128, 2], F32)
            nc.gpsimd.dma_start(b1sb[:], b1d[:])
            b2sb = cpool.tile([128, 1], F32)
            nc.gpsimd.dma_start(b2sb[:], b2d[:])
            b3sb = cpool.tile([128, 2], F32)
            nc.gpsimd.dma_start(b3sb[:], b3d[:])
            if bias_mm:
                b4row = cpool.tile([1, D], F32R)
                nc.gpsimd.dma_start(b4row[:], b4d[:])
                ones_row = cpool.tile([1, 128], F32R)
                onesr_f = cpool.tile([1, 128], F32)
                nc.vector.memset(onesr_f[:], 1.0)
                nc.scalar.copy(ones_row[:], onesr_f[:])

            # ---------------- per-slab stage emitters ----------------
            def emit_p1(s, m):
                d = st[s]
                if "hT" not in d:
                    d["hT"] = hT_pool.tile([128, 1024], BF16, tag="hT",
                                           name=f"hT_{s}")
                ph = pbig.tile([128, 512], F32, tag="big", name=f"ph{m}_{s}")
                for c in range(7):
                    nc.tensor.matmul(
                        ph[:],
                        W1sb[:, c * H + m * 128: c * H + m * 128 + 128],
                        d["xa"][:, c * SLAB:(c + 1) * SLAB],
                        start=(c == 0), stop=(c == 6),
                    )
                # relu(ph + b1) on DVE (ACT is kept clear for the topk exps
                # so the attention chain isn't queue-blocked)
                nc.vector.tensor_scalar(
                    out=d["hT"][:, m * 512:(m + 1) * 512], in0=ph[:],
                    scalar1=b1sb[:, m:m + 1], scalar2=0.0,
                    op0=ALU.add, op1=ALU.max,
                )

            def emit_p2(s):
                d = st[s]
                pz = pmisc.tile([128, 512], F32, tag="misc", name=f"pz_{s}")
                for m in range(2):
                    nc.tensor.matmul(
                        pz[:], W2sb[:, m * 128:(m + 1) * 128],
                        d["hT"][:, m * 512:(m + 1) * 512],
                        start=(m == 0), stop=(m == 1),
                    )
                d["zT"] = zT_pool.tile([128, SLAB], BF16, tag="zT",
                                       name=f"zT_{s}")
                # z = pz + b2 (GpSimd can't read PSUM; Identity is in the
                # exp table set so this costs no table reload)
                nc.scalar.activation(d["zT"][:], pz[:], AF.Identity,
                                     bias=b2sb[:, 0:1])

            def emit_norm(s):
                # squared z on GpSimd (SBUF->SBUF); row sums come from PE
                # matmuls against a ones column (emit_sims)
                d = st[s]
                d["zsqT"] = sm_pool.tile([128, 512], BF16, tag="zsqT",
                                         name=f"zsqT_{s}")
                nc.gpsimd.tensor_tensor(d["zsqT"][:], d["zT"][:], d["zT"][:],
                                        ALU.mult)

            def emit_sims(s):
                d = st[s]
                psim = pmisc.tile([128, 512], F32, tag="misc",
                                  name=f"psim_{s}")
                for t in range(4):
                    nc.tensor.matmul(
                        psim[:, t * M:(t + 1) * M],
                        d["zT"][:, t * 128:(t + 1) * 128], mnT[:],
                        start=True, stop=True,
                    )
                for t in range(4):
                    # ||z||^2 for row tile t -> psim col 200+t (needs the
                    # GpSimd square; emitted after the sims MMs so the PE
                    # isn't blocked on it)
                    nc.tensor.matmul(
                        psim[:, 4 * M + t:4 * M + t + 1],
                        d["zsqT"][:, t * 128:(t + 1) * 128], ones_col[:],
                        start=True, stop=True,
                    )
                d["psim"] = psim
                # inv = 1/sqrt(nsq): magic-constant seed + Newton steps.
                # seed/h read PSUM on DVE; Newton runs on GpSimd from SBUF.
                nsq = psim[:, 4 * M:4 * M + 4]
                seed_i = sm_pool.tile([128, 4], I32, tag="seed_i",
                                      name=f"seed_i_{s}")
                nc.vector.tensor_scalar(
                    out=seed_i[:], in0=nsq.bitcast(I32),
                    scalar1=1, scalar2=None, op0=ALU.logical_shift_right,
                )
                y0_i = sm_pool.tile([128, 4], I32, tag="y0_i",
                                    name=f"y0_i_{s}")
                nc.vector.tensor_scalar(
                    out=y0_i[:], in0=seed_i[:],
                    scalar1=-1, scalar2=0x5F3759DF, op0=ALU.mult,
                    op1=ALU.add,
                )
                h = sm_pool.tile([128, 4], F32, tag="h", name=f"h_{s}")
                nc.vector.tensor_scalar(
                    out=h[:], in0=nsq, scalar1=0.5, scalar2=1e-30,
                    op0=ALU.mult, op1=ALU.max,
                )
                ycur = y0_i[:].bitcast(F32)
                for it in range(NEWTON_STEPS):
                    a = sm_pool.tile([128, 4], F32, tag=f"nta{it}",
                                     name=f"nta{it}_{s}")
                    nc.gpsimd.tensor_tensor(a[:], ycur, ycur, ALU.mult)
                    b_ = sm_pool.tile([128, 4], F32, tag=f"ntb{it}",
                                      name=f"ntb{it}_{s}")
                    nc.gpsimd.tensor_tensor(b_[:], a[:], h[:], ALU.mult)
                    c_ = sm_pool.tile([128, 4], F32, tag=f"ntc{it}",
                                      name=f"ntc{it}_{s}")
                    nc.gpsimd.tensor_scalar(
                        out=c_[:], in0=b_[:], scalar1=-1.0, scalar2=1.5,
                        op0=ALU.mult, op1=ALU.add,
                    )
                    ynext = sm_pool.tile([128, 4], F32, tag=f"nty{it}",
                                         name=f"nty{it}_{s}")
                    nc.gpsimd.tensor_tensor(ynext[:], ycur, c_[:], ALU.mult)
                    ycur = ynext[:]
                d["invcol"] = ycur

            def emit_topk(s):
                d = st[s]
                # top-k thresholding runs on RAW sims (scale-invariant);
                # 1/||z|| is folded into the Exp's per-partition scale
                simsb = sm_pool.tile([128, 4 * M], F32, tag="simsb",
                                     name=f"simsb_{s}")
                nc.vector.tensor_copy(simsb[:], d["psim"][:, :4 * M])
                m8 = sm_pool.tile([128, 32], F32, tag="m8", name=f"m8_{s}")
                msk = sm_pool.tile([128, 4 * M], F32, tag="msk",
                                   name=f"msk_{s}")
                for t in range(4):
                    nc.vector.max(m8[:, t * 8:(t + 1) * 8],
                                  simsb[:, t * M:(t + 1) * M])
                    nc.vector.scalar_tensor_tensor(
                        out=msk[:, t * M:(t + 1) * M],
                        in0=simsb[:, t * M:(t + 1) * M],
                        scalar=m8[:, t * 8 + 4:t * 8 + 5],
                        in1=simsb[:, t * M:(t + 1) * M],
                        op0=ALU.is_ge, op1=ALU.mult,
                    )
                # per-row-tile exp -> denominator accum -> recip -> scale so
                # tile t's tail overlaps tile t+1's exp (exp(0)=1 for masked
                # entries, as in reference)
                pexp = sm_pool.tile([128, 4 * M], F32, tag="pexp",
                                    name=f"pexp_{s}")
                den = sm_pool.tile([128, 4], F32, tag="den", name=f"den_{s}")
                rden = sm_pool.tile([128, 4], F32, tag="rden",
                                    name=f"rden_{s}")
                attn = sm_pool.tile([128, 4 * M], BF16, tag="attn",
                                    name=f"attn_{s}")
                for t in range(4):
                    nc.scalar.activation(
                        pexp[:, t * M:(t + 1) * M], msk[:, t * M:(t + 1) * M],
                        AF.Exp, scale=d["invcol"][:, t:t + 1],
                        accum_out=den[:, t:t + 1],
                    )
                    nc.vector.reciprocal(rden[:, t:t + 1], den[:, t:t + 1])
                    nc.vector.tensor_scalar_mul(
                        attn[:, t * M:(t + 1) * M],
                        pexp[:, t * M:(t + 1) * M],
                        rden[:, t:t + 1],
                    )
                d["attn"] = attn

            def emit_pat(s):
                d = st[s]
                pat = pmisc.tile([128, 512], BF16, tag="misc",
                                 name=f"pat_{s}")
                for t in range(4):
                    nc.tensor.transpose(
                        pat[:M, t * 128:(t + 1) * 128],
                        d["attn"][:, t * M:(t + 1) * M], ident_b[:],
                    )
                attnT = sm_pool.tile([M, SLAB], BF16, tag="attnT",
                                     name=f"attnT_{s}")
                nc.vector.tensor_copy(attnT[:], pat[:M, :])
                d["attnT"] = attnT

            def emit_dec(s):
                d = st[s]
                d["dT"] = hT_pool.tile([128, 1024], BF16, tag="dT",
                                       name=f"dT_{s}")
                for m in range(2):
                    pd = pbig.tile([128, 512], F32, tag="big",
                                   name=f"pd{m}_{s}")
                    nc.tensor.matmul(
                        pd[:], W3msb[:, m * 128:(m + 1) * 128], d["attnT"][:],
                        start=True, stop=True,
                    )
                    # relu(pd + b3) -> bf16 dT; split DVE/ACT for balance
                    if m == 0:
                        nc.vector.tensor_scalar(
                            out=d["dT"][:, m * 512:(m + 1) * 512], in0=pd[:],
                            scalar1=b3sb[:, m:m + 1], scalar2=0.0,
                            op0=ALU.add, op1=ALU.max,
                        )
                    else:
                        nc.scalar.activation(
                            d["dT"][:, m * 512:(m + 1) * 512], pd[:],
                            AF.Relu, bias=b3sb[:, m:m + 1],
                        )

            def emit_final(s, tiles):
                d = st[s]
                if "xo" not in d:
                    d["xo"] = xo_pool.tile([128, 4, D], BF16, tag="xo",
                                           name=f"xo_{s}")
                for t in tiles:
                    # two 1-bank PSUM tiles per row tile (nh halves); each
                    # dT stationary chunk loaded once for both halves
                    pxs = [pxp.tile([128, 512], F32, tag="x",
                                    name=f"px{t}{nh}_{s}") for nh in range(2)]
                    if bias_mm:
                        for nh in range(2):
                            nc.tensor.matmul(
                                pxs[nh][:, :NHALF], ones_row[:],
                                b4row[:, nh * NHALF:(nh + 1) * NHALF],
                                start=True, stop=False,
                            )
                    for k in range(2):
                        for nh in range(2):
                            nc.tensor.matmul(
                                pxs[nh][:, :NHALF],
                                d["dT"][:, k * 512 + t * 128:
                                        k * 512 + t * 128 + 128],
                                W4sb[:, k * D + nh * NHALF:
                                     k * D + (nh + 1) * NHALF],
                                start=(k == 0 and not bias_mm), stop=(k == 1),
                            )
                    # sigmoid(v) = 0.5*tanh(0.5 v)+0.5; tanh on ACT,
                    # affine fixup on GpSimd, bf16 out
                    for nh in range(2):
                        nc.scalar.activation(
                            d["xo"][:, t, nh * NHALF:(nh + 1) * NHALF],
                            pxs[nh][:, :NHALF], AF.Tanh, scale=0.5,
                        )
                    nc.gpsimd.tensor_scalar(
                        out=d["xo"][:, t, :], in0=d["xo"][:, t, :],
                        scalar1=0.5, scalar2=0.5,
                        op0=ALU.mult, op1=ALU.add,
                    )

            def emit_out(s):
                nc.sync.dma_start(y_r[s], st[s]["xo"][:])
                del st[s]

            def emit_keepalive(s):
                # dep-free 1-column matmul into an unused psim column: keeps
                # the PE HAM activity window from going idle at the block
                # boundary (re-throttle to 1.2GHz costs ~2.5us/slab)
                if s in st and "psim" in st[s]:
                    nc.tensor.matmul(
                        st[s]["psim"][0:1, 205:206], ones_col[0:1, :],
                        ones_col[0:1, :], start=True, stop=True,
                    )

            # ------------- depth-3 software-pipelined slab loop -------------
            # Block s runs: encoder of slab s (PE), topk chain of s-1
            # (DVE/ACT), attention tail + decoder of s-2 (PE). The topk
            # chain thus has a full block of PE work to hide behind.
            for s in range(n_slabs):
                if s >= 2:
                    emit_pat(s - 2)
                emit_keepalive(s - 1)
                emit_p1(s, 0)
                emit_p1(s, 1)
                if s >= 2:
                    emit_dec(s - 2)
                    emit_final(s - 2, [0, 1])
                emit_p2(s)
                if s >= 1:
                    emit_topk(s - 1)
                emit_norm(s)
                emit_sims(s)
                if s + 2 < n_slabs:
                    emit_dma_in(s + 2)
                if s >= 2:
                    emit_final(s - 2, [2, 3])
                emit_keepalive(s)
                if s >= 2:
                    emit_out(s - 2)
            n = n_slabs
            emit_topk(n - 1)
            emit_pat(n - 2)
            emit_dec(n - 2)
            emit_final(n - 2, [0, 1])
            emit_pat(n - 1)
            emit_dec(n - 1)
            emit_final(n - 2, [2, 3])
            emit_out(n - 2)
            emit_final(n - 1, [0, 1, 2, 3])
            emit_out(n - 1)

    nc.finalize()
    return nc


_cache: dict = {}


def _get_nc(rows: int, n_cores: int, bias_mm: bool):
    key = (rows, n_cores, bias_mm)
    if key not in _cache:
        _cache[key] = _build(rows, n_cores, bias_mm)
    return _cache[key]


def kernel(**inputs):
    x = np.asarray(inputs["x"], dtype=np.float32)
    rows = x.shape[0]
    n_cores = N_CORES
    rows_pc = rows // n_cores
    n_slabs = rows_pc // SLAB
    b4 = np.asarray(inputs["b4"], np.float32)
    bias_mm = not np.allclose(b4, 0.0)
    nc = _get_nc(rows_pc, n_cores, bias_mm)

    W1 = np.asarray(inputs["W1"], np.float32)
    W2 = np.asarray(inputs["W2"], np.float32)
    W3 = np.asarray(inputs["W3"], np.float32)
    W4 = np.asarray(inputs["W4"], np.float32)
    mem = np.asarray(inputs["memory"], np.float32)
    b1 = np.asarray(inputs["b1"], np.float32)
    b2 = np.asarray(inputs["b2"], np.float32)
    b3 = np.asarray(inputs["b3"], np.float32)

    # pre-packed weight layouts (see _build)
    W1sb = np.ascontiguousarray(
        W1.reshape(7, PCH, H).transpose(1, 0, 2).reshape(PCH, 7 * H)
    ).astype(NPBF)
    W2sb = np.ascontiguousarray(
        W2.reshape(2, 128, E).transpose(1, 0, 2).reshape(128, H)
    ).astype(NPBF)
    nrm = np.sqrt((mem * mem).sum(axis=1, keepdims=True))
    mn = mem / np.maximum(nrm, 1e-12)
    mnT = np.ascontiguousarray(mn.T).astype(NPBF)               # [128, 50]
    W3m = (mem.astype(np.float64) @ W3.astype(np.float64)).astype(NPBF)
    W4sb = np.ascontiguousarray(
        W4.reshape(2, 128, D).transpose(1, 0, 2).reshape(128, 2 * D)
    ).astype(NPBF)
    b1c = np.ascontiguousarray(b1.reshape(2, 128).T)            # [128, 2]
    b2c = np.ascontiguousarray(b2.reshape(128, 1))
    b3c = np.ascontiguousarray(b3.reshape(2, 128).T)

    weights = {
        "W1sb": W1sb, "W2sb": W2sb, "mnT": mnT,
        "W3msb": W3m, "W4sb": W4sb, "b1c": b1c, "b2c": b2c, "b3c": b3c,
    }
    if bias_mm:
        weights["b4row"] = np.ascontiguousarray(b4.reshape(1, D))

    # per-core x, host-transposed into the slab layout:
    # xmain[s, p, c*512 + r] = x[s*512 + r, c*112 + p],  c in [0, 7)
    xb16 = x.astype(NPBF)
    in_maps = []
    for c in range(n_cores):
        xc = xb16[c * rows_pc:(c + 1) * rows_pc].reshape(n_slabs, SLAB, D)
        xmain = np.ascontiguousarray(
            xc.reshape(n_slabs, SLAB, 7, PCH).transpose(0, 3, 2, 1)
        ).reshape(n_slabs, PCH, 7 * SLAB)
        in_maps.append({"xmain": xmain, **weights})

    res = run_bass_kernel_spmd(
        nc, in_maps, list(range(n_cores)), trace=TRACE
    )
    kernel.last_result = res
    y = np.concatenate([np.asarray(res.results[c]["y"])
                        for c in range(n_cores)], axis=0)
    return y.astype(np.float32)
